# revision 2
# baseline (speedup 1.0000x reference)
"""Trainium2 Bass kernel for a Mix9Net-style directional CNN (v4).

Network (per image, 4 directions d with unit vectors u_d):
  xs[d] = silu(dconv3(x, w_d0, b_d0, u_d))                      # Cin=2 -> 128
  4x DirectionalConvResBlock:
      t = silu(dconv3(xs[d], res_wd[l], res_bd[l], u_d))        # 128 -> 128
      t = silu(conv1x1(t, res_w1[l], res_b1[l]))                # 128 -> 128
      xs[d] = t + xs[d]
  Conv0dResBlock:
      xs[d] = silu(conv1x1(silu(conv1x1(xs[d], c0_w1, c0_b1)), c0_w2, c0_b2)) + xs[d]
  out[d] = conv1x1(xs[d], wf, bf)                               # 128 -> 64
Output stacked: [B, 4, 64, 15, 15].

ScalarE (ACT) is the bottleneck: silu runs at 1 elem/cycle/lane (1.2 GHz,
dtype-independent), so the payload floor is 11 evals x 4 dirs x 450 px x
64 groups / 1.2GHz ~= 1.06 ms/core; every design choice minimizes ACT
instruction count (engine APs are limited to 3 free dims) and keeps
PE (~1.0ms) / DVE (~0.9ms) underneath while preserving overlap:

  - Activations: channels on partitions, pixels on the free dim, padded
    flat layout (16-col row pitch, 16-row image pitch + guards) so the 4
    directional 3-tap convs are 3 shifted fp32r matmuls PSUM-accumulated.
  - PSUM pair tiles [128, 2dirs, 512] (ring of 4): drains at 2-bank
    granularity keep ScalarE queued while PE fills other tiles.
  - PSUM image blocks are 15 rows x 16 (no pad row), so (img, row) merges
    into one uniform 30-row dim and a pair drain is a single legal
    [2, 30, 15] ACT op (900 real px, pad column stripped).
  - t1/t2/u1/u2 intermediates are compact [128, 4, 450] (contiguous, even
    innermost for fp32r) so 1x1 convs and their drains carry no pads.
  - Initial Cin=2 conv is one K=6 matmul per direction: GPSIMD-queue DMAs
    pre-stage 3 tap-shifted copies of the zero-padded input stream into
    partitions (tap, ch) of [6, 4, XLEN], prefetched one group ahead.
  - LANES independent image-pair pipelines hide each lane's serial
    drain -> residual-add -> next-conv chain.

Sharding: pure data parallel, batch 1024 -> 128 images on each of 8 cores.
"""

import numpy as np

import concourse.bacc as bacc
import concourse.tile as tile
from concourse import mybir
from concourse import bass_utils

F32 = mybir.dt.float32
SILU = mybir.ActivationFunctionType.Silu

# geometry
H = 15
RP = 16                     # row pitch: 15 data cols + 1 zero pad col
IMGB = RP * RP              # 256: 15 data rows + 1 zero pad row
GROUP = 2                   # images per matmul
PAY = GROUP * IMGB          # 512
GUARD = 32                  # flat guard so shifted reads stay in bounds
XLEN = GUARD + PAY + GUARD  # 576
NMM = GROUP * H * RP        # 480 matmul columns (incl. pad cols)
NOUT = GROUP * H * H        # 450 real pixels
NB = 512                    # fp32 elements per PSUM bank
NRES = 4
DM = 128
DOUT = 64
CIN = 2
NCORES = 8
BATCH = 1024
BPC = BATCH // NCORES       # images per core
XGW = 656                   # host-side padded stream width per group
XGOFF = 64                  # image payload offset within the stream
DIRS = ((0, 1), (1, 0), (1, 1), (1, -1))
OFFS = [dy * RP + dx for (dy, dx) in DIRS]  # flat offsets: 1, 16, 17, 15

N_LANES = 5


def _rhs(t, d, off):
    """[128, GROUP, 15, 16] tap-conv rhs: 15 data rows, full 16-wide."""
    v = t[:, d, GUARD + off: GUARD + off + PAY]
    return v.rearrange("p (i r c) -> p i r c", i=GROUP, r=RP, c=RP)[:, :, :H, :]


def _xsd(t, d):
    """[128, GROUP, 15, 15] data interior of direction d of padded xs."""
    v = t[:, d, GUARD: GUARD + PAY]
    return v.rearrange("p (i r c) -> p i r c", i=GROUP, r=RP, c=RP)[:, :, :H, :H]


def _w240(t, d):
    """[128, GROUP, 240] contiguous 15 rows x 16 cols per image."""
    v = t[:, d, GUARD: GUARD + PAY]
    return v.rearrange("p (i q) -> p i q", i=GROUP, q=IMGB)[:, :, :H * RP]


def _pstrip(ps, nd):
    """[P, nd, 30, 15] pad-col-stripped view of a [P, nd, NB] psum tile."""
    v = ps[:, :, :NMM].rearrange("p d (q c) -> p d q c", q=GROUP * H, c=RP)
    return v[:, :, :, :H]


def _c30(t, d0, nd):
    """[P, nd, 30, 15] view of dirs [d0, d0+nd) of a compact [P, 4, 450] tile."""
    return t[:, d0: d0 + nd, :].rearrange(
        "p d (q c) -> p d q c", q=GROUP * H, c=H)


def _cd(t, d):
    """[P, GROUP, 15, 15] view of direction d of a compact [P, 4, 450] tile."""
    return t[:, d, :].rearrange("p (i r c) -> p i r c", i=GROUP, r=H, c=H)


def build_nc(n_imgs, lanes=N_LANES, mm_dt=mybir.dt.float32r, pairs=True,
             enable_asserts=False):
    ng = n_imgs // GROUP
    nc = bacc.Bacc(
        "TRN2",
        target_bir_lowering=False,
        debug=False,
        enable_asserts=enable_asserts,
    )
    DT = mm_dt
    xg_d = nc.dram_tensor("xg", (CIN, ng, XGW), DT, kind="ExternalInput")
    w06_d = nc.dram_tensor("w06", (6, DM), DT, kind="ExternalInput")
    wd_d = nc.dram_tensor("wdT", (NRES, 3, DM, DM), DT, kind="ExternalInput")
    w1_d = nc.dram_tensor("w1T", (NRES, DM, DM), DT, kind="ExternalInput")
    c0_d = nc.dram_tensor("c0wT", (2, DM, DM), DT, kind="ExternalInput")
    wf_d = nc.dram_tensor("wfT", (DM, DM), DT, kind="ExternalInput")
    b_d = nc.dram_tensor("biases", (DM, 12), F32, kind="ExternalInput")
    out_d = nc.dram_tensor("out", (n_imgs, 4, DOUT, H, H), F32, kind="ExternalOutput")

    LANES = min(lanes, max(ng, 1))
    ND = 2 if pairs else 4          # directions per PSUM tile
    NPAIR = 4 // ND                 # tiles per stage

    with tile.TileContext(nc) as tc:
        with (
            tc.tile_pool(name="consts", bufs=1) as consts,
            tc.tile_pool(name="tmp", bufs=LANES) as tmp,
            tc.tile_pool(name="outp", bufs=3) as outp,
            tc.tile_pool(name="psp", bufs=8 // ND, space="PSUM") as psp,
        ):
            w06_sb = consts.tile([6, DM], DT, tag="w06")
            wd_sb = consts.tile([DM, NRES, 3, DM], DT, tag="wd")
            w1_sb = consts.tile([DM, NRES, DM], DT, tag="w1")
            c0_sb = consts.tile([DM, 2, DM], DT, tag="c0")
            wf_sb = consts.tile([DM, DM], DT, tag="wf")
            bias_sb = consts.tile([DM, 12], F32, tag="bias")
            nc.sync.dma_start(out=w06_sb, in_=w06_d.ap())
            nc.sync.dma_start(out=wd_sb, in_=wd_d.ap().rearrange("l k i o -> i l k o"))
            nc.sync.dma_start(out=w1_sb, in_=w1_d.ap().rearrange("l i o -> i l o"))
            nc.sync.dma_start(out=c0_sb, in_=c0_d.ap().rearrange("t i o -> i t o"))
            nc.sync.dma_start(out=wf_sb, in_=wf_d.ap())
            nc.sync.dma_start(out=bias_sb, in_=b_d.ap())

            # persistent per-lane activation state; pad cols/rows and guards
            # are zeroed once and only 15x15 interiors written afterwards,
            # preserving conv zero-padding semantics.
            xs_bufs = [
                consts.tile([DM, 4, XLEN], DT, tag=f"xs{i}", name=f"xs{i}")
                for i in range(LANES)
            ]
            # tap-shifted input copies on partitions (tap, ch); every window
            # is fully rewritten by DMA each group (zero guards come from
            # the host stream).
            xp6_bufs = [
                consts.tile([6, 4, XLEN], DT, tag=f"xp{i}", name=f"xp{i}")
                for i in range(LANES)
            ]
            # zero-fill via uint32 bitcast: walrus has no f32r memset
            # encoding, and 0 is bit-identical across formats.
            for t in xs_bufs:
                nc.vector.memset(t.bitcast(mybir.dt.uint32), 0)
            for t in xp6_bufs:
                nc.vector.memset(t.bitcast(mybir.dt.uint32), 0)

            xg_v = xg_d.ap()
            out_v = out_d.ap().rearrange("b d o h w -> o d b (h w)")

            def stage_x(g):
                """Stage the 12 tap-shifted padded input windows of group g
                (issued from the idle GPSIMD queue, prefetched a group ahead)."""
                xp6 = xp6_bufs[g % LANES]
                for d in range(4):
                    for k in range(3):
                        off = (k - 1) * OFFS[d]
                        src0 = XGOFF - GUARD + off
                        nc.gpsimd.dma_start(
                            out=xp6[2 * k: 2 * k + 2, d, :],
                            in_=xg_v[:, g, src0: src0 + XLEN],
                        )

            def group_stages(g):
                xs = xs_bufs[g % LANES]
                xp6 = xp6_bufs[g % LANES]
                i0 = g * GROUP

                def ptile(nm, j):
                    return psp.tile([DM, ND, NB], F32, tag="ps",
                                    name=f"{nm}{g}_{j}")

                def conv_stage(nm, n_cols, emit_mm, emit_sink):
                    """For each PSUM tile: fill dirs [j*ND, (j+1)*ND) via
                    emit_mm(psview, dd, d, n_cols), then emit_sink(ps, j)."""
                    for j in range(NPAIR):
                        ps = ptile(nm, j)
                        for dd in range(ND):
                            emit_mm(ps, dd, j * ND + dd, n_cols)
                        emit_sink(ps, j)

                def s_init():
                    def mm(ps, dd, d, n):
                        v = xp6[:, d, GUARD: GUARD + PAY]
                        v = v.rearrange("p (i r c) -> p i r c", i=GROUP, r=RP, c=RP)
                        nc.tensor.matmul(ps[:, dd, :n], w06_sb, v[:, :, :H, :],
                                         start=True, stop=True)
                    def sink(ps, j):
                        # dest is the padded xs interior: per-direction ops
                        # (image pitch 256 != 240 blocks the (i,r) merge)
                        sv = _pstrip(ps, ND)
                        for dd in range(ND):
                            d = j * ND + dd
                            src = sv[:, dd].rearrange(
                                "p (i r) c -> p i r c", i=GROUP, r=H)
                            nc.scalar.activation(
                                _xsd(xs, d), src, SILU, bias=bias_sb[:, 0:1],
                            )
                    conv_stage("psI", NMM, mm, sink)
                yield s_init

                for l in range(NRES):
                    def s_taps(l=l):
                        t1 = tmp.tile([DM, 4, NOUT], DT, tag="t1",
                                      name=f"t1_{g}_{l}")
                        def mm(ps, dd, d, n):
                            for k in range(3):
                                off = (k - 1) * OFFS[d]
                                nc.tensor.matmul(
                                    ps[:, dd, :n], wd_sb[:, l, k, :],
                                    _rhs(xs, d, off),
                                    start=(k == 0), stop=(k == 2),
                                )
                        def sink(ps, j):
                            nc.scalar.activation(
                                _c30(t1, j * ND, ND), _pstrip(ps, ND), SILU,
                                bias=bias_sb[:, 1 + l: 2 + l],
                            )
                        conv_stage(f"psA{l}_", NMM, mm, sink)
                        stage_out[0] = t1
                    def s_mix(l=l):
                        t1 = stage_out[0]
                        t2 = tmp.tile([DM, 4, NOUT], DT, tag="t2",
                                      name=f"t2_{g}_{l}")
                        def mm(ps, dd, d, n):
                            nc.tensor.matmul(ps[:, dd, :n], w1_sb[:, l, :],
                                             t1[:, d, :], start=True, stop=True)
                        def sink(ps, j):
                            nc.scalar.activation(
                                t2[:, j * ND: (j + 1) * ND, :],
                                ps[:, :, :NOUT], SILU,
                                bias=bias_sb[:, 5 + l: 6 + l],
                            )
                            for dd in range(ND):
                                d = j * ND + dd
                                xi = _xsd(xs, d)
                                nc.vector.tensor_add(xi, xi, _cd(t2, d))
                        conv_stage(f"psB{l}_", NOUT, mm, sink)
                    stage_out = [None]
                    yield s_taps
                    yield s_mix

                def s_c0a():
                    u1 = tmp.tile([DM, 4, NOUT], DT, tag="t1", name=f"u1_{g}")
                    def mm(ps, dd, d, n):
                        nc.tensor.matmul(ps[:, dd, :n], c0_sb[:, 0, :],
                                         _w240(xs, d), start=True, stop=True)
                    def sink(ps, j):
                        nc.scalar.activation(
                            _c30(u1, j * ND, ND), _pstrip(ps, ND), SILU,
                            bias=bias_sb[:, 9:10],
                        )
                    conv_stage("psC", NMM, mm, sink)
                    c0_out[0] = u1
                    if g + LANES < ng:
                        stage_x(g + LANES)
                c0_out = [None]
                yield s_c0a

                def s_c0b():
                    u1 = c0_out[0]
                    u2 = tmp.tile([DM, 4, NOUT], DT, tag="t2", name=f"u2_{g}")
                    def mm(ps, dd, d, n):
                        nc.tensor.matmul(ps[:, dd, :n], c0_sb[:, 1, :],
                                         u1[:, d, :], start=True, stop=True)
                    def sink(ps, j):
                        nc.scalar.activation(
                            u2[:, j * ND: (j + 1) * ND, :], ps[:, :, :NOUT],
                            SILU, bias=bias_sb[:, 10:11],
                        )
                        for dd in range(ND):
                            d = j * ND + dd
                            ud = _cd(u2, d)
                            nc.vector.tensor_add(ud, ud, _xsd(xs, d))
                    conv_stage("psD", NOUT, mm, sink)
                    c0_out[0] = u2
                yield s_c0b

                def s_final():
                    u2 = c0_out[0]
                    ob = outp.tile([DOUT, 4, NOUT], F32, tag="ob", name=f"ob{g}")
                    def mm(ps, dd, d, n):
                        nc.tensor.matmul(ps[:, dd, :n], wf_sb, u2[:, d, :],
                                         start=True, stop=True)
                    def sink(ps, j):
                        nc.vector.tensor_scalar_add(
                            ob[:, j * ND: (j + 1) * ND, :],
                            ps[:DOUT, :, :NOUT], bias_sb[:DOUT, 11:12],
                        )
                    conv_stage("psF", NOUT, mm, sink)
                    ob_v = ob.rearrange("o d (i p) -> o d i p", i=GROUP)
                    for i in range(GROUP):
                        nc.sync.dma_start(
                            out=out_v[:, :, i0 + i, :], in_=ob_v[:, :, i, :]
                        )
                yield s_final

            lane_groups = [[] for _ in range(LANES)]
            for g in range(ng):
                lane_groups[g % LANES].append(g)

            for g in range(min(LANES, ng)):
                stage_x(g)

            def lane_stream(groups):
                for g in groups:
                    yield from group_stages(g)

            streams = [lane_stream(gs) for gs in lane_groups]
            # Skew lanes so same-kind stages (12 per group) never align.
            for li, s in enumerate(streams):
                prime = 3 * (LANES - 1 - li)
                for _ in range(prime):
                    stage = next(s, None)
                    if stage is not None:
                        stage()
            while streams:
                nxt = []
                for s in streams:
                    stage = next(s, None)
                    if stage is not None:
                        stage()
                        nxt.append(s)
                streams = nxt

    nc.compile()
    return nc


def prep_weights(w_d0, res_wd, res_w1, c0_w1, c0_w2, wf,
                 b_d0, res_bd, res_b1, c0_b1, c0_b2, bf):
    f = lambda a: np.ascontiguousarray(np.asarray(a), dtype=np.float32)
    w06 = np.zeros((6, DM), np.float32)                   # rows (tap, ch)
    for k in range(3):
        for c in range(CIN):
            w06[2 * k + c] = np.asarray(w_d0)[:, c, k]
    wdT = f(np.asarray(res_wd).transpose(0, 3, 2, 1))     # [l, k, ci, co]
    w1T = f(np.asarray(res_w1).transpose(0, 2, 1))        # [l, ci, co]
    c0wT = f(np.stack([np.asarray(c0_w1).T, np.asarray(c0_w2).T]))
    wfT = np.zeros((DM, DM), np.float32)                  # pad 64 -> 128 cols
    wfT[:, :DOUT] = np.asarray(wf).T
    biases = np.zeros((DM, 12), np.float32)
    biases[:, 0] = np.asarray(b_d0)
    for l in range(NRES):
        biases[:, 1 + l] = np.asarray(res_bd)[l]
        biases[:, 5 + l] = np.asarray(res_b1)[l]
    biases[:, 9] = np.asarray(c0_b1)
    biases[:, 10] = np.asarray(c0_b2)
    biases[:DOUT, 11] = np.asarray(bf)
    return dict(w06=w06, wdT=wdT, w1T=w1T, c0wT=c0wT, wfT=wfT, biases=biases)


def prep_x(x):
    """[B, 2, 15, 15] -> padded stream [CIN, B//2, XGW]: each group's two
    images as 16x16 zero-padded blocks at [XGOFF, XGOFF+512)."""
    B = x.shape[0]
    xb = np.zeros((B, CIN, RP, RP), np.float32)
    xb[:, :, :H, :H] = x
    xb = xb.reshape(B // GROUP, GROUP, CIN, IMGB)
    xg = np.zeros((B // GROUP, CIN, XGW), np.float32)
    xg[:, :, XGOFF: XGOFF + PAY] = (
        xb.transpose(0, 2, 1, 3).reshape(B // GROUP, CIN, PAY))
    return np.ascontiguousarray(xg.transpose(1, 0, 2))


_NC_CACHE = {}


def _get_nc():
    if "nc" not in _NC_CACHE:
        _NC_CACHE["nc"] = build_nc(BPC)
    return _NC_CACHE["nc"]


def kernel(x, w_d0, b_d0, res_wd, res_bd, res_w1, res_b1,
           c0_w1, c0_b1, c0_w2, c0_b2, wf, bf, _trace=False):
    x = np.ascontiguousarray(np.asarray(x), dtype=np.float32)
    assert x.shape == (BATCH, CIN, H, H), x.shape
    w = prep_weights(w_d0, res_wd, res_w1, c0_w1, c0_w2, wf,
                     b_d0, res_bd, res_b1, c0_b1, c0_b2, bf)
    xg = prep_x(x)
    ngc = BPC // GROUP
    nc = _get_nc()
    in_maps = [
        dict(xg=np.ascontiguousarray(xg[:, c * ngc:(c + 1) * ngc]), **w)
        for c in range(NCORES)
    ]
    res = bass_utils.run_bass_kernel_spmd(
        nc, in_maps, core_ids=list(range(NCORES)), trace=_trace
    )
    out = np.concatenate([r["out"] for r in res.results], axis=0)
    if _trace:
        return out, res
    return out


# revision 3
# speedup vs baseline: 1.7748x; 1.7748x over previous
"""Trainium2 Bass kernel for a Mix9Net-style directional CNN (v4).

Network (per image, 4 directions d with unit vectors u_d):
  xs[d] = silu(dconv3(x, w_d0, b_d0, u_d))                      # Cin=2 -> 128
  4x DirectionalConvResBlock:
      t = silu(dconv3(xs[d], res_wd[l], res_bd[l], u_d))        # 128 -> 128
      t = silu(conv1x1(t, res_w1[l], res_b1[l]))                # 128 -> 128
      xs[d] = t + xs[d]
  Conv0dResBlock:
      xs[d] = silu(conv1x1(silu(conv1x1(xs[d], c0_w1, c0_b1)), c0_w2, c0_b2)) + xs[d]
  out[d] = conv1x1(xs[d], wf, bf)                               # 128 -> 64
Output stacked: [B, 4, 64, 15, 15].

ScalarE (ACT) is the bottleneck: silu runs at 1 elem/cycle/lane (1.2 GHz,
dtype-independent), so the payload floor is 11 evals x 4 dirs x 450 px x
64 groups / 1.2GHz ~= 1.06 ms/core; every design choice minimizes ACT
instruction count (engine APs are limited to 3 free dims) and keeps
PE (~1.0ms) / DVE (~0.9ms) underneath while preserving overlap:

  - Activations: channels on partitions, pixels on the free dim, padded
    flat layout (16-col row pitch, 16-row image pitch + guards) so the 4
    directional 3-tap convs are 3 shifted fp32r matmuls PSUM-accumulated.
  - PSUM pair tiles [128, 2dirs, 512] (ring of 4): drains at 2-bank
    granularity keep ScalarE queued while PE fills other tiles.
  - PSUM image blocks are 15 rows x 16 (no pad row), so (img, row) merges
    into one uniform 30-row dim and a pair drain is a single legal
    [2, 30, 15] ACT op (900 real px, pad column stripped).
  - t1/t2/u1/u2 intermediates are compact [128, 4, 450] (contiguous, even
    innermost for fp32r) so 1x1 convs and their drains carry no pads.
  - Initial Cin=2 conv is one K=6 matmul per direction: GPSIMD-queue DMAs
    pre-stage 3 tap-shifted copies of the zero-padded input stream into
    partitions (tap, ch) of [6, 4, XLEN], prefetched one group ahead.
  - LANES independent image-pair pipelines hide each lane's serial
    drain -> residual-add -> next-conv chain.

Sharding: pure data parallel, batch 1024 -> 128 images on each of 8 cores.
"""

import numpy as np

import concourse.bacc as bacc
import concourse.tile as tile
from concourse import mybir
from concourse import bass_utils

F32 = mybir.dt.float32
SILU = mybir.ActivationFunctionType.Silu

# geometry
H = 15
RP = 16                     # row pitch: 15 data cols + 1 zero pad col
IMGB = RP * RP              # 256: 15 data rows + 1 zero pad row
GROUP = 2                   # images per matmul
PAY = GROUP * IMGB          # 512
GUARD = 32                  # flat guard so shifted reads stay in bounds
XLEN = GUARD + PAY + GUARD  # 576
NMM = GROUP * H * RP        # 480 matmul columns (incl. pad cols)
NOUT = GROUP * H * H        # 450 real pixels
NB = 512                    # fp32 elements per PSUM bank
NRES = 4
DM = 128
DOUT = 64
CIN = 2
NCORES = 8
BATCH = 1024
BPC = BATCH // NCORES       # images per core
XGW = 656                   # host-side padded stream width per group
XGOFF = 64                  # image payload offset within the stream
DIRS = ((0, 1), (1, 0), (1, 1), (1, -1))
OFFS = [dy * RP + dx for (dy, dx) in DIRS]  # flat offsets: 1, 16, 17, 15

N_LANES = 5


def _rhs(t, d, off):
    """[128, GROUP, 15, 16] tap-conv rhs: 15 data rows, full 16-wide."""
    v = t[:, d, GUARD + off: GUARD + off + PAY]
    return v.rearrange("p (i r c) -> p i r c", i=GROUP, r=RP, c=RP)[:, :, :H, :]


def _xsd(t, d):
    """[128, GROUP, 15, 15] data interior of direction d of padded xs."""
    v = t[:, d, GUARD: GUARD + PAY]
    return v.rearrange("p (i r c) -> p i r c", i=GROUP, r=RP, c=RP)[:, :, :H, :H]


def _w240(t, d):
    """[128, GROUP, 240] contiguous 15 rows x 16 cols per image."""
    v = t[:, d, GUARD: GUARD + PAY]
    return v.rearrange("p (i q) -> p i q", i=GROUP, q=IMGB)[:, :, :H * RP]


def _pstrip(ps, nd):
    """[P, nd, 30, 15] pad-col-stripped view of a [P, nd, NB] psum tile."""
    v = ps[:, :, :NMM].rearrange("p d (q c) -> p d q c", q=GROUP * H, c=RP)
    return v[:, :, :, :H]


def _c30(t, d0, nd):
    """[P, nd, 30, 15] view of dirs [d0, d0+nd) of a compact [P, 4, 450] tile."""
    return t[:, d0: d0 + nd, :].rearrange(
        "p d (q c) -> p d q c", q=GROUP * H, c=H)


def _cd(t, d):
    """[P, GROUP, 15, 15] view of direction d of a compact [P, 4, 450] tile."""
    return t[:, d, :].rearrange("p (i r c) -> p i r c", i=GROUP, r=H, c=H)


def build_nc(n_imgs, lanes=N_LANES, mm_dt=mybir.dt.float32r, pairs=True,
             enable_asserts=False):
    ng = n_imgs // GROUP
    nc = bacc.Bacc(
        "TRN2",
        target_bir_lowering=False,
        debug=False,
        enable_asserts=enable_asserts,
    )
    DT = mm_dt
    xg_d = nc.dram_tensor("xg", (CIN, ng, XGW), DT, kind="ExternalInput")
    w06_d = nc.dram_tensor("w06", (6, DM), DT, kind="ExternalInput")
    wd_d = nc.dram_tensor("wdT", (NRES, 3, DM, DM), DT, kind="ExternalInput")
    w1_d = nc.dram_tensor("w1T", (NRES, DM, DM), DT, kind="ExternalInput")
    c0_d = nc.dram_tensor("c0wT", (2, DM, DM), DT, kind="ExternalInput")
    wf_d = nc.dram_tensor("wfT", (DM, DM), DT, kind="ExternalInput")
    b_d = nc.dram_tensor("biases", (DM, 12), F32, kind="ExternalInput")
    out_d = nc.dram_tensor("out", (n_imgs, 4, DOUT, H, H), F32, kind="ExternalOutput")

    LANES = min(lanes, max(ng, 1))
    ND = 2 if pairs else 4          # directions per PSUM tile
    NPAIR = 4 // ND                 # tiles per stage

    with tile.TileContext(nc) as tc:
        with (
            tc.tile_pool(name="consts", bufs=1) as consts,
            tc.tile_pool(name="tmp", bufs=LANES) as tmp,
            tc.tile_pool(name="outp", bufs=3) as outp,
            tc.tile_pool(name="psp", bufs=8 // ND, space="PSUM") as psp,
        ):
            w06_sb = consts.tile([6, DM], DT, tag="w06")
            wd_sb = consts.tile([DM, NRES, 3, DM], DT, tag="wd")
            w1_sb = consts.tile([DM, NRES, DM], DT, tag="w1")
            c0_sb = consts.tile([DM, 2, DM], DT, tag="c0")
            wf_sb = consts.tile([DM, DM], DT, tag="wf")
            bias_sb = consts.tile([DM, 12], F32, tag="bias")
            nc.sync.dma_start(out=w06_sb, in_=w06_d.ap())
            nc.sync.dma_start(out=wd_sb, in_=wd_d.ap().rearrange("l k i o -> i l k o"))
            nc.sync.dma_start(out=w1_sb, in_=w1_d.ap().rearrange("l i o -> i l o"))
            nc.sync.dma_start(out=c0_sb, in_=c0_d.ap().rearrange("t i o -> i t o"))
            nc.sync.dma_start(out=wf_sb, in_=wf_d.ap())
            nc.sync.dma_start(out=bias_sb, in_=b_d.ap())

            # persistent per-lane activation state; pad cols/rows and guards
            # are zeroed once and only 15x15 interiors written afterwards,
            # preserving conv zero-padding semantics.
            xs_bufs = [
                consts.tile([DM, 4, XLEN], DT, tag=f"xs{i}", name=f"xs{i}")
                for i in range(LANES)
            ]
            # tap-shifted input copies on partitions (tap, ch); every window
            # is fully rewritten by DMA each group (zero guards come from
            # the host stream).
            xp6_bufs = [
                consts.tile([6, 4, XLEN], DT, tag=f"xp{i}", name=f"xp{i}")
                for i in range(LANES)
            ]
            # zero-fill via uint32 bitcast: walrus has no f32r memset
            # encoding, and 0 is bit-identical across formats. xp6 needs no
            # scrub: every XLEN window is fully DMA-rewritten (guards
            # included, from the zero-padded host stream) before each use.
            for t in xs_bufs:
                nc.vector.memset(t.bitcast(mybir.dt.uint32), 0)

            xg_v = xg_d.ap()
            out_v = out_d.ap().rearrange("b d o h w -> o d b (h w)")

            def stage_x(g):
                """Stage the 12 tap-shifted padded input windows of group g
                (issued from the idle GPSIMD queue, prefetched a group ahead)."""
                xp6 = xp6_bufs[g % LANES]
                for d in range(4):
                    for k in range(3):
                        off = (k - 1) * OFFS[d]
                        src0 = XGOFF - GUARD + off
                        nc.gpsimd.dma_start(
                            out=xp6[2 * k: 2 * k + 2, d, :],
                            in_=xg_v[:, g, src0: src0 + XLEN],
                        )

            def group_stages(g):
                xs = xs_bufs[g % LANES]
                xp6 = xp6_bufs[g % LANES]
                i0 = g * GROUP

                def ptile(nm, j):
                    return psp.tile([DM, ND, NB], F32, tag="ps",
                                    name=f"{nm}{g}_{j}")

                def conv_stage(nm, n_cols, emit_mm, emit_sink):
                    """For each PSUM tile: fill dirs [j*ND, (j+1)*ND) via
                    emit_mm(psview, dd, d, n_cols), then emit_sink(ps, j)."""
                    for j in range(NPAIR):
                        ps = ptile(nm, j)
                        for dd in range(ND):
                            emit_mm(ps, dd, j * ND + dd, n_cols)
                        emit_sink(ps, j)

                def s_init():
                    def mm(ps, dd, d, n):
                        v = xp6[:, d, GUARD: GUARD + PAY]
                        v = v.rearrange("p (i r c) -> p i r c", i=GROUP, r=RP, c=RP)
                        nc.tensor.matmul(ps[:, dd, :n], w06_sb, v[:, :, :H, :],
                                         start=True, stop=True)
                    def sink(ps, j):
                        # dest is the padded xs interior: per-direction ops
                        # (image pitch 256 != 240 blocks the (i,r) merge)
                        sv = _pstrip(ps, ND)
                        for dd in range(ND):
                            d = j * ND + dd
                            src = sv[:, dd].rearrange(
                                "p (i r) c -> p i r c", i=GROUP, r=H)
                            nc.scalar.activation(
                                _xsd(xs, d), src, SILU, bias=bias_sb[:, 0:1],
                            )
                    conv_stage("psI", NMM, mm, sink)
                yield s_init

                for l in range(NRES):
                    def s_taps(l=l):
                        t1 = tmp.tile([DM, 4, NOUT], DT, tag="t1",
                                      name=f"t1_{g}_{l}")
                        def mm(ps, dd, d, n):
                            for k in range(3):
                                off = (k - 1) * OFFS[d]
                                nc.tensor.matmul(
                                    ps[:, dd, :n], wd_sb[:, l, k, :],
                                    _rhs(xs, d, off),
                                    start=(k == 0), stop=(k == 2),
                                )
                        def sink(ps, j):
                            nc.scalar.activation(
                                _c30(t1, j * ND, ND), _pstrip(ps, ND), SILU,
                                bias=bias_sb[:, 1 + l: 2 + l],
                            )
                        conv_stage(f"psA{l}_", NMM, mm, sink)
                        stage_out[0] = t1
                    def s_mix(l=l):
                        t1 = stage_out[0]
                        t2 = tmp.tile([DM, 4, NOUT], DT, tag="t2",
                                      name=f"t2_{g}_{l}")
                        def mm(ps, dd, d, n):
                            nc.tensor.matmul(ps[:, dd, :n], w1_sb[:, l, :],
                                             t1[:, d, :], start=True, stop=True)
                        def sink(ps, j):
                            nc.scalar.activation(
                                t2[:, j * ND: (j + 1) * ND, :],
                                ps[:, :, :NOUT], SILU,
                                bias=bias_sb[:, 5 + l: 6 + l],
                            )
                            for dd in range(ND):
                                d = j * ND + dd
                                xi = _xsd(xs, d)
                                nc.vector.tensor_add(xi, xi, _cd(t2, d))
                        conv_stage(f"psB{l}_", NOUT, mm, sink)
                    stage_out = [None]
                    yield s_taps
                    yield s_mix

                def s_c0a():
                    u1 = tmp.tile([DM, 4, NOUT], DT, tag="t1", name=f"u1_{g}")
                    def mm(ps, dd, d, n):
                        nc.tensor.matmul(ps[:, dd, :n], c0_sb[:, 0, :],
                                         _w240(xs, d), start=True, stop=True)
                    def sink(ps, j):
                        nc.scalar.activation(
                            _c30(u1, j * ND, ND), _pstrip(ps, ND), SILU,
                            bias=bias_sb[:, 9:10],
                        )
                    conv_stage("psC", NMM, mm, sink)
                    c0_out[0] = u1
                    if g + LANES < ng:
                        stage_x(g + LANES)
                c0_out = [None]
                yield s_c0a

                def s_c0b():
                    u1 = c0_out[0]
                    u2 = tmp.tile([DM, 4, NOUT], DT, tag="t2", name=f"u2_{g}")
                    def mm(ps, dd, d, n):
                        nc.tensor.matmul(ps[:, dd, :n], c0_sb[:, 1, :],
                                         u1[:, d, :], start=True, stop=True)
                    def sink(ps, j):
                        nc.scalar.activation(
                            u2[:, j * ND: (j + 1) * ND, :], ps[:, :, :NOUT],
                            SILU, bias=bias_sb[:, 10:11],
                        )
                        for dd in range(ND):
                            d = j * ND + dd
                            ud = _cd(u2, d)
                            nc.vector.tensor_add(ud, ud, _xsd(xs, d))
                    conv_stage("psD", NOUT, mm, sink)
                    c0_out[0] = u2
                yield s_c0b

                def s_final():
                    u2 = c0_out[0]
                    ob = outp.tile([DOUT, 4, NOUT], F32, tag="ob", name=f"ob{g}")
                    def mm(ps, dd, d, n):
                        nc.tensor.matmul(ps[:, dd, :n], wf_sb, u2[:, d, :],
                                         start=True, stop=True)
                    def sink(ps, j):
                        nc.vector.tensor_scalar_add(
                            ob[:, j * ND: (j + 1) * ND, :],
                            ps[:DOUT, :, :NOUT], bias_sb[:DOUT, 11:12],
                        )
                    conv_stage("psF", NOUT, mm, sink)
                    ob_v = ob.rearrange("o d (i p) -> o d i p", i=GROUP)
                    for i in range(GROUP):
                        nc.sync.dma_start(
                            out=out_v[:, :, i0 + i, :], in_=ob_v[:, :, i, :]
                        )
                yield s_final

            lane_groups = [[] for _ in range(LANES)]
            for g in range(ng):
                lane_groups[g % LANES].append(g)

            for g in range(min(LANES, ng)):
                stage_x(g)

            def lane_stream(groups):
                for g in groups:
                    yield from group_stages(g)

            streams = [lane_stream(gs) for gs in lane_groups]
            # Skew lanes so same-kind stages (12 per group) never align.
            for li, s in enumerate(streams):
                prime = 3 * (LANES - 1 - li)
                for _ in range(prime):
                    stage = next(s, None)
                    if stage is not None:
                        stage()
            while streams:
                nxt = []
                for s in streams:
                    stage = next(s, None)
                    if stage is not None:
                        stage()
                        nxt.append(s)
                streams = nxt

    nc.compile()
    return nc


def prep_weights(w_d0, res_wd, res_w1, c0_w1, c0_w2, wf,
                 b_d0, res_bd, res_b1, c0_b1, c0_b2, bf):
    f = lambda a: np.ascontiguousarray(np.asarray(a), dtype=np.float32)
    w06 = np.zeros((6, DM), np.float32)                   # rows (tap, ch)
    for k in range(3):
        for c in range(CIN):
            w06[2 * k + c] = np.asarray(w_d0)[:, c, k]
    wdT = f(np.asarray(res_wd).transpose(0, 3, 2, 1))     # [l, k, ci, co]
    w1T = f(np.asarray(res_w1).transpose(0, 2, 1))        # [l, ci, co]
    c0wT = f(np.stack([np.asarray(c0_w1).T, np.asarray(c0_w2).T]))
    wfT = np.zeros((DM, DM), np.float32)                  # pad 64 -> 128 cols
    wfT[:, :DOUT] = np.asarray(wf).T
    biases = np.zeros((DM, 12), np.float32)
    biases[:, 0] = np.asarray(b_d0)
    for l in range(NRES):
        biases[:, 1 + l] = np.asarray(res_bd)[l]
        biases[:, 5 + l] = np.asarray(res_b1)[l]
    biases[:, 9] = np.asarray(c0_b1)
    biases[:, 10] = np.asarray(c0_b2)
    biases[:DOUT, 11] = np.asarray(bf)
    return dict(w06=w06, wdT=wdT, w1T=w1T, c0wT=c0wT, wfT=wfT, biases=biases)


def prep_x(x):
    """[B, 2, 15, 15] -> padded stream [CIN, B//2, XGW]: each group's two
    images as 16x16 zero-padded blocks at [XGOFF, XGOFF+512)."""
    B = x.shape[0]
    xb = np.zeros((B, CIN, RP, RP), np.float32)
    xb[:, :, :H, :H] = x
    xb = xb.reshape(B // GROUP, GROUP, CIN, IMGB)
    xg = np.zeros((B // GROUP, CIN, XGW), np.float32)
    xg[:, :, XGOFF: XGOFF + PAY] = (
        xb.transpose(0, 2, 1, 3).reshape(B // GROUP, CIN, PAY))
    return np.ascontiguousarray(xg.transpose(1, 0, 2))


_NC_CACHE = {}


def _get_nc():
    if "nc" not in _NC_CACHE:
        _NC_CACHE["nc"] = build_nc(BPC)
    return _NC_CACHE["nc"]


def kernel(x, w_d0, b_d0, res_wd, res_bd, res_w1, res_b1,
           c0_w1, c0_b1, c0_w2, c0_b2, wf, bf, _trace=False):
    x = np.ascontiguousarray(np.asarray(x), dtype=np.float32)
    assert x.shape == (BATCH, CIN, H, H), x.shape
    w = prep_weights(w_d0, res_wd, res_w1, c0_w1, c0_w2, wf,
                     b_d0, res_bd, res_b1, c0_b1, c0_b2, bf)
    xg = prep_x(x)
    ngc = BPC // GROUP
    nc = _get_nc()
    in_maps = [
        dict(xg=np.ascontiguousarray(xg[:, c * ngc:(c + 1) * ngc]), **w)
        for c in range(NCORES)
    ]
    res = bass_utils.run_bass_kernel_spmd(
        nc, in_maps, core_ids=list(range(NCORES)), trace=_trace
    )
    out = np.concatenate([r["out"] for r in res.results], axis=0)
    if _trace:
        return out, res
    return out


# revision 4
# speedup vs baseline: 2.5581x; 1.4414x over previous
"""Trainium2 Bass kernel for a Mix9Net-style directional CNN (v4).

Network (per image, 4 directions d with unit vectors u_d):
  xs[d] = silu(dconv3(x, w_d0, b_d0, u_d))                      # Cin=2 -> 128
  4x DirectionalConvResBlock:
      t = silu(dconv3(xs[d], res_wd[l], res_bd[l], u_d))        # 128 -> 128
      t = silu(conv1x1(t, res_w1[l], res_b1[l]))                # 128 -> 128
      xs[d] = t + xs[d]
  Conv0dResBlock:
      xs[d] = silu(conv1x1(silu(conv1x1(xs[d], c0_w1, c0_b1)), c0_w2, c0_b2)) + xs[d]
  out[d] = conv1x1(xs[d], wf, bf)                               # 128 -> 64
Output stacked: [B, 4, 64, 15, 15].

ScalarE (ACT) is the bottleneck: silu runs at 1 elem/cycle/lane (1.2 GHz,
dtype-independent), so the payload floor is 11 evals x 4 dirs x 450 px x
64 groups / 1.2GHz ~= 1.06 ms/core; every design choice minimizes ACT
instruction count (engine APs are limited to 3 free dims) and keeps
PE (~1.0ms) / DVE (~0.9ms) underneath while preserving overlap:

  - Activations: channels on partitions, pixels on the free dim, padded
    flat layout (16-col row pitch, 16-row image pitch + guards) so the 4
    directional 3-tap convs are 3 shifted fp32r matmuls PSUM-accumulated.
  - PSUM pair tiles [128, 2dirs, 512] (ring of 4): drains at 2-bank
    granularity keep ScalarE queued while PE fills other tiles.
  - PSUM image blocks are 15 rows x 16 (no pad row), so (img, row) merges
    into one uniform 30-row dim and a pair drain is a single legal
    [2, 30, 15] ACT op (900 real px, pad column stripped).
  - t1/t2/u1/u2 intermediates are compact [128, 4, 450] (contiguous, even
    innermost for fp32r) so 1x1 convs and their drains carry no pads.
  - Initial Cin=2 conv is one K=6 matmul per direction: GPSIMD-queue DMAs
    pre-stage 3 tap-shifted copies of the zero-padded input stream into
    partitions (tap, ch) of [6, 4, XLEN], prefetched one group ahead.
  - LANES independent image-pair pipelines hide each lane's serial
    drain -> residual-add -> next-conv chain.

Sharding: pure data parallel, batch 1024 -> 128 images on each of 8 cores.
"""

import numpy as np

import concourse.bacc as bacc
import concourse.tile as tile
from concourse import mybir
from concourse import bass_utils

F32 = mybir.dt.float32
SILU = mybir.ActivationFunctionType.Silu

# geometry
H = 15
RP = 16                     # row pitch: 15 data cols + 1 zero pad col
IMGB = RP * RP              # 256: 15 data rows + 1 zero pad row
GROUP = 2                   # images per matmul
PAY = GROUP * IMGB          # 512
GUARD = 32                  # flat guard so shifted reads stay in bounds
XLEN = GUARD + PAY + GUARD  # 576
NMM = GROUP * H * RP        # 480 matmul columns (incl. pad cols)
NOUT = GROUP * H * H        # 450 real pixels
NB = 512                    # fp32 elements per PSUM bank
NRES = 4
DM = 128
DOUT = 64
CIN = 2
NCORES = 8
BATCH = 1024
BPC = BATCH // NCORES       # images per core
XGW = 656                   # host-side padded stream width per group
XGOFF = 64                  # image payload offset within the stream
DIRS = ((0, 1), (1, 0), (1, 1), (1, -1))
OFFS = [dy * RP + dx for (dy, dx) in DIRS]  # flat offsets: 1, 16, 17, 15

N_LANES = 5


def _rhs(t, d, off):
    """[128, GROUP, 15, 16] tap-conv rhs: 15 data rows, full 16-wide."""
    v = t[:, d, GUARD + off: GUARD + off + PAY]
    return v.rearrange("p (i r c) -> p i r c", i=GROUP, r=RP, c=RP)[:, :, :H, :]


def _xsd(t, d):
    """[128, GROUP, 15, 15] data interior of direction d of padded xs."""
    v = t[:, d, GUARD: GUARD + PAY]
    return v.rearrange("p (i r c) -> p i r c", i=GROUP, r=RP, c=RP)[:, :, :H, :H]


def _w240(t, d):
    """[128, GROUP, 240] contiguous 15 rows x 16 cols per image."""
    v = t[:, d, GUARD: GUARD + PAY]
    return v.rearrange("p (i q) -> p i q", i=GROUP, q=IMGB)[:, :, :H * RP]


def _pstrip(ps, nd):
    """[P, nd, 30, 15] pad-col-stripped view of a [P, nd, NB] psum tile."""
    v = ps[:, :, :NMM].rearrange("p d (q c) -> p d q c", q=GROUP * H, c=RP)
    return v[:, :, :, :H]


def _c30(t, d0, nd):
    """[P, nd, 30, 15] view of dirs [d0, d0+nd) of a compact [P, 4, 450] tile."""
    return t[:, d0: d0 + nd, :].rearrange(
        "p d (q c) -> p d q c", q=GROUP * H, c=H)


def _cd(t, d):
    """[P, GROUP, 15, 15] view of direction d of a compact [P, 4, 450] tile."""
    return t[:, d, :].rearrange("p (i r c) -> p i r c", i=GROUP, r=H, c=H)


def build_nc(n_imgs, lanes=N_LANES, mm_dt=mybir.dt.float32r, pairs=True,
             enable_asserts=False):
    ng = n_imgs // GROUP
    nc = bacc.Bacc(
        "TRN2",
        target_bir_lowering=False,
        debug=False,
        enable_asserts=enable_asserts,
    )
    DT = mm_dt
    xg_d = nc.dram_tensor("xg", (CIN, ng, XGW), DT, kind="ExternalInput")
    w06_d = nc.dram_tensor("w06", (6, DM), DT, kind="ExternalInput")
    wd_d = nc.dram_tensor("wdT", (NRES, 3, DM, DM), DT, kind="ExternalInput")
    w1_d = nc.dram_tensor("w1T", (NRES, DM, DM), DT, kind="ExternalInput")
    c0_d = nc.dram_tensor("c0wT", (2, DM, DM), DT, kind="ExternalInput")
    wf_d = nc.dram_tensor("wfT", (DM, DM), DT, kind="ExternalInput")
    b_d = nc.dram_tensor("biases", (DM, 12), F32, kind="ExternalInput")
    out_d = nc.dram_tensor("out", (n_imgs, 4, DOUT, H, H), F32, kind="ExternalOutput")

    LANES = min(lanes, max(ng, 1))
    ND = 2 if pairs else 4          # directions per PSUM tile
    NPAIR = 4 // ND                 # tiles per stage

    with tile.TileContext(nc) as tc:
        with (
            tc.tile_pool(name="consts", bufs=1) as consts,
            tc.tile_pool(name="tmp", bufs=LANES) as tmp,
            tc.tile_pool(name="outp", bufs=3) as outp,
            tc.tile_pool(name="psp", bufs=8 // ND, space="PSUM") as psp,
        ):
            w06_sb = consts.tile([6, DM], DT, tag="w06")
            wd_sb = consts.tile([DM, NRES, 3, DM], DT, tag="wd")
            w1_sb = consts.tile([DM, NRES, DM], DT, tag="w1")
            c0_sb = consts.tile([DM, 2, DM], DT, tag="c0")
            wf_sb = consts.tile([DM, DM], DT, tag="wf")
            bias_sb = consts.tile([DM, 12], F32, tag="bias")
            nc.sync.dma_start(out=w06_sb, in_=w06_d.ap())
            nc.sync.dma_start(out=wd_sb, in_=wd_d.ap().rearrange("l k i o -> i l k o"))
            nc.sync.dma_start(out=w1_sb, in_=w1_d.ap().rearrange("l i o -> i l o"))
            nc.sync.dma_start(out=c0_sb, in_=c0_d.ap().rearrange("t i o -> i t o"))
            nc.sync.dma_start(out=wf_sb, in_=wf_d.ap())
            nc.sync.dma_start(out=bias_sb, in_=b_d.ap())

            # persistent per-lane activation state; pad cols/rows and guards
            # are zeroed once and only 15x15 interiors written afterwards,
            # preserving conv zero-padding semantics.
            xs_bufs = [
                consts.tile([DM, 4, XLEN], DT, tag=f"xs{i}", name=f"xs{i}")
                for i in range(LANES)
            ]
            # tap-shifted input copies on partitions (tap, ch); every window
            # is fully rewritten by DMA each group (zero guards come from
            # the host stream).
            xp6_bufs = [
                consts.tile([6, 4, XLEN], DT, tag=f"xp{i}", name=f"xp{i}")
                for i in range(LANES)
            ]
            # zero-fill via uint32 bitcast: walrus has no f32r memset
            # encoding, and 0 is bit-identical across formats. xp6 needs no
            # scrub: every XLEN window is fully DMA-rewritten (guards
            # included, from the zero-padded host stream) before each use.
            for t in xs_bufs:
                nc.vector.memset(t.bitcast(mybir.dt.uint32), 0)

            xg_v = xg_d.ap()
            out_v = out_d.ap().rearrange("b d o h w -> o d b (h w)")

            def stage_x(g):
                """Stage the 12 tap-shifted padded input windows of group g
                (issued from the idle GPSIMD queue, prefetched a group ahead)."""
                xp6 = xp6_bufs[g % LANES]
                for d in range(4):
                    for k in range(3):
                        off = (k - 1) * OFFS[d]
                        src0 = XGOFF - GUARD + off
                        nc.gpsimd.dma_start(
                            out=xp6[2 * k: 2 * k + 2, d, :],
                            in_=xg_v[:, g, src0: src0 + XLEN],
                        )

            def group_stages(g):
                xs = xs_bufs[g % LANES]
                xp6 = xp6_bufs[g % LANES]
                i0 = g * GROUP

                def ptile(nm, j):
                    return psp.tile([DM, ND, NB], F32, tag="ps",
                                    name=f"{nm}{g}_{j}")

                def conv_stage(nm, n_cols, emit_mm, emit_sink):
                    """For each PSUM tile: fill dirs [j*ND, (j+1)*ND) via
                    emit_mm(psview, dd, d, n_cols), then emit_sink(ps, j)."""
                    for j in range(NPAIR):
                        ps = ptile(nm, j)
                        for dd in range(ND):
                            emit_mm(ps, dd, j * ND + dd, n_cols)
                        emit_sink(ps, j)

                def s_init():
                    # drain compact (one big ACT op per tile; the padded-xs
                    # image pitch 256 != 240 would force per-direction ops),
                    # then DVE copies the interiors into padded xs.
                    x0 = tmp.tile([DM, 4, NOUT], DT, tag="t2", name=f"x0_{g}")
                    def mm(ps, dd, d, n):
                        v = xp6[:, d, GUARD: GUARD + PAY]
                        v = v.rearrange("p (i r c) -> p i r c", i=GROUP, r=RP, c=RP)
                        nc.tensor.matmul(ps[:, dd, :n], w06_sb, v[:, :, :H, :],
                                         start=True, stop=True)
                    def sink(ps, j):
                        nc.scalar.activation(
                            _c30(x0, j * ND, ND), _pstrip(ps, ND), SILU,
                            bias=bias_sb[:, 0:1],
                        )
                        for dd in range(ND):
                            d = j * ND + dd
                            nc.vector.tensor_copy(_xsd(xs, d), _cd(x0, d))
                    conv_stage("psI", NMM, mm, sink)
                yield s_init

                for l in range(NRES):
                    def s_taps(l=l):
                        t1 = tmp.tile([DM, 4, NOUT], DT, tag="t1",
                                      name=f"t1_{g}_{l}")
                        def mm(ps, dd, d, n):
                            for k in range(3):
                                off = (k - 1) * OFFS[d]
                                nc.tensor.matmul(
                                    ps[:, dd, :n], wd_sb[:, l, k, :],
                                    _rhs(xs, d, off),
                                    start=(k == 0), stop=(k == 2),
                                )
                        def sink(ps, j):
                            nc.scalar.activation(
                                _c30(t1, j * ND, ND), _pstrip(ps, ND), SILU,
                                bias=bias_sb[:, 1 + l: 2 + l],
                            )
                        conv_stage(f"psA{l}_", NMM, mm, sink)
                        stage_out[0] = t1
                    def s_mix(l=l):
                        t1 = stage_out[0]
                        t2 = tmp.tile([DM, 4, NOUT], DT, tag="t2",
                                      name=f"t2_{g}_{l}")
                        def mm(ps, dd, d, n):
                            nc.tensor.matmul(ps[:, dd, :n], w1_sb[:, l, :],
                                             t1[:, d, :], start=True, stop=True)
                        def sink(ps, j):
                            nc.scalar.activation(
                                t2[:, j * ND: (j + 1) * ND, :],
                                ps[:, :, :NOUT], SILU,
                                bias=bias_sb[:, 5 + l: 6 + l],
                            )
                            for dd in range(ND):
                                d = j * ND + dd
                                xi = _xsd(xs, d)
                                nc.vector.tensor_add(xi, xi, _cd(t2, d))
                        conv_stage(f"psB{l}_", NOUT, mm, sink)
                    stage_out = [None]
                    yield s_taps
                    yield s_mix

                def s_c0a():
                    u1 = tmp.tile([DM, 4, NOUT], DT, tag="t1", name=f"u1_{g}")
                    def mm(ps, dd, d, n):
                        nc.tensor.matmul(ps[:, dd, :n], c0_sb[:, 0, :],
                                         _w240(xs, d), start=True, stop=True)
                    def sink(ps, j):
                        nc.scalar.activation(
                            _c30(u1, j * ND, ND), _pstrip(ps, ND), SILU,
                            bias=bias_sb[:, 9:10],
                        )
                    conv_stage("psC", NMM, mm, sink)
                    c0_out[0] = u1
                    if g + LANES < ng:
                        stage_x(g + LANES)
                c0_out = [None]
                yield s_c0a

                def s_c0b():
                    u1 = c0_out[0]
                    u2 = tmp.tile([DM, 4, NOUT], DT, tag="t2", name=f"u2_{g}")
                    def mm(ps, dd, d, n):
                        nc.tensor.matmul(ps[:, dd, :n], c0_sb[:, 1, :],
                                         u1[:, d, :], start=True, stop=True)
                    def sink(ps, j):
                        nc.scalar.activation(
                            u2[:, j * ND: (j + 1) * ND, :], ps[:, :, :NOUT],
                            SILU, bias=bias_sb[:, 10:11],
                        )
                        for dd in range(ND):
                            d = j * ND + dd
                            ud = _cd(u2, d)
                            nc.vector.tensor_add(ud, ud, _xsd(xs, d))
                    conv_stage("psD", NOUT, mm, sink)
                    c0_out[0] = u2
                yield s_c0b

                def s_final():
                    u2 = c0_out[0]
                    ob = outp.tile([DOUT, 4, NOUT], F32, tag="ob", name=f"ob{g}")
                    def mm(ps, dd, d, n):
                        nc.tensor.matmul(ps[:, dd, :n], wf_sb, u2[:, d, :],
                                         start=True, stop=True)
                    def sink(ps, j):
                        nc.vector.tensor_scalar_add(
                            ob[:, j * ND: (j + 1) * ND, :],
                            ps[:DOUT, :, :NOUT], bias_sb[:DOUT, 11:12],
                        )
                    conv_stage("psF", NOUT, mm, sink)
                    ob_v = ob.rearrange("o d (i p) -> o d i p", i=GROUP)
                    for i in range(GROUP):
                        nc.sync.dma_start(
                            out=out_v[:, :, i0 + i, :], in_=ob_v[:, :, i, :]
                        )
                yield s_final

            lane_groups = [[] for _ in range(LANES)]
            for g in range(ng):
                lane_groups[g % LANES].append(g)

            for g in range(min(LANES, ng)):
                stage_x(g)

            def lane_stream(groups):
                for g in groups:
                    yield from group_stages(g)

            streams = [lane_stream(gs) for gs in lane_groups]
            # Skew lanes so same-kind stages (12 per group) never align.
            for li, s in enumerate(streams):
                prime = 3 * (LANES - 1 - li)
                for _ in range(prime):
                    stage = next(s, None)
                    if stage is not None:
                        stage()
            while streams:
                nxt = []
                for s in streams:
                    stage = next(s, None)
                    if stage is not None:
                        stage()
                        nxt.append(s)
                streams = nxt

    nc.compile()
    return nc


def prep_weights(w_d0, res_wd, res_w1, c0_w1, c0_w2, wf,
                 b_d0, res_bd, res_b1, c0_b1, c0_b2, bf):
    f = lambda a: np.ascontiguousarray(np.asarray(a), dtype=np.float32)
    w06 = np.zeros((6, DM), np.float32)                   # rows (tap, ch)
    for k in range(3):
        for c in range(CIN):
            w06[2 * k + c] = np.asarray(w_d0)[:, c, k]
    wdT = f(np.asarray(res_wd).transpose(0, 3, 2, 1))     # [l, k, ci, co]
    w1T = f(np.asarray(res_w1).transpose(0, 2, 1))        # [l, ci, co]
    c0wT = f(np.stack([np.asarray(c0_w1).T, np.asarray(c0_w2).T]))
    wfT = np.zeros((DM, DM), np.float32)                  # pad 64 -> 128 cols
    wfT[:, :DOUT] = np.asarray(wf).T
    biases = np.zeros((DM, 12), np.float32)
    biases[:, 0] = np.asarray(b_d0)
    for l in range(NRES):
        biases[:, 1 + l] = np.asarray(res_bd)[l]
        biases[:, 5 + l] = np.asarray(res_b1)[l]
    biases[:, 9] = np.asarray(c0_b1)
    biases[:, 10] = np.asarray(c0_b2)
    biases[:DOUT, 11] = np.asarray(bf)
    return dict(w06=w06, wdT=wdT, w1T=w1T, c0wT=c0wT, wfT=wfT, biases=biases)


def prep_x(x):
    """[B, 2, 15, 15] -> padded stream [CIN, B//2, XGW]: each group's two
    images as 16x16 zero-padded blocks at [XGOFF, XGOFF+512)."""
    B = x.shape[0]
    xb = np.zeros((B, CIN, RP, RP), np.float32)
    xb[:, :, :H, :H] = x
    xb = xb.reshape(B // GROUP, GROUP, CIN, IMGB)
    xg = np.zeros((B // GROUP, CIN, XGW), np.float32)
    xg[:, :, XGOFF: XGOFF + PAY] = (
        xb.transpose(0, 2, 1, 3).reshape(B // GROUP, CIN, PAY))
    return np.ascontiguousarray(xg.transpose(1, 0, 2))


_NC_CACHE = {}


def _get_nc():
    if "nc" not in _NC_CACHE:
        _NC_CACHE["nc"] = build_nc(BPC)
    return _NC_CACHE["nc"]


def kernel(x, w_d0, b_d0, res_wd, res_bd, res_w1, res_b1,
           c0_w1, c0_b1, c0_w2, c0_b2, wf, bf, _trace=False):
    x = np.ascontiguousarray(np.asarray(x), dtype=np.float32)
    assert x.shape == (BATCH, CIN, H, H), x.shape
    w = prep_weights(w_d0, res_wd, res_w1, c0_w1, c0_w2, wf,
                     b_d0, res_bd, res_b1, c0_b1, c0_b2, bf)
    xg = prep_x(x)
    ngc = BPC // GROUP
    nc = _get_nc()
    in_maps = [
        dict(xg=np.ascontiguousarray(xg[:, c * ngc:(c + 1) * ngc]), **w)
        for c in range(NCORES)
    ]
    res = bass_utils.run_bass_kernel_spmd(
        nc, in_maps, core_ids=list(range(NCORES)), trace=_trace
    )
    out = np.concatenate([r["out"] for r in res.results], axis=0)
    if _trace:
        return out, res
    return out
